# revision 8
# baseline (speedup 1.0000x reference)
"""Distributed GATv1 (2x GAT + SAGE + MLP head) for Trainium2, 8 NeuronCores.

Strategy (graph/data parallel, per sharding hint):
- Nodes are sharded contiguously across the 8 cores; each core's local nodes
  are re-binned into tiles of 128 ("dst bins") balanced by in-degree so every
  bin has nearly the same number of incoming edges.
- Per GAT layer: a sharded dense phase computes g = [h | al_src] and al_dst
  for local nodes, then an AllGather replicates g; the edge phase gathers
  g[src] rows with indirect DMA, computes softmax weights
  w = exp(leaky_relu(al_s + al_d)) per edge (numerically safe without the
  max-subtraction since |logits| are O(1)), scales the gathered rows, and
  aggregates messages per dst bin with a one-hot "routing" matmul that also
  accumulates the softmax denominators as 3 extra columns.
- SAGE mean-aggregation reuses the same machinery with unit weights; its
  linear layers and the whole MLP head collapse into two [192,16] matmuls
  (no nonlinearity in between), folded on the host.
"""

import numpy as np

# Problem constants (hardcoded; kernel.py must be self-contained).
N = 50000
E = 800000
IN_C = 128
HID = 64
HEADS = 3
OUT_C = 16
C = HEADS * HID          # 192
ROW = C + HEADS          # 195 = [h | al_s]
NCORES = 8
P = 128


def _ceil(a, b):
    return -(-a - 0) // b if False else -(-a // b)


def _pack_bins(deg, nbins):
    """Greedy balanced binning: assign n=nbins*128 nodes to bins of 128 slots,
    minimizing the max per-bin edge count. Returns (bin_of, slot_of)."""
    n = len(deg)
    assert n == nbins * P
    order = np.argsort(-deg, kind="stable")
    bin_load = np.zeros(nbins, np.int64)
    bin_fill = np.zeros(nbins, np.int64)
    bin_of = np.zeros(n, np.int32)
    slot_of = np.zeros(n, np.int32)
    big = np.int64(1 << 60)
    for l in order:
        cand = np.where(bin_fill < P, bin_load, big)
        b = int(np.argmin(cand))
        bin_of[l] = b
        slot_of[l] = bin_fill[b]
        bin_fill[b] += 1
        bin_load[b] += deg[l]
    assert (bin_fill == P).all()
    return bin_of, slot_of


def _bucket_edges(e_src_pg, e_dstperm, nbins, int_extra=None):
    """Bucket edges by dst bin into [nbins, P, T] arrays (T = max needed).
    Returns (T, src_a[nbins,P,T] i32, slot_a[nbins,P,T] f32, extra_a or None)."""
    ebin = e_dstperm // P
    eslot = (e_dstperm % P).astype(np.float32)
    counts = np.bincount(ebin, minlength=nbins)
    T = max(1, _ceil(int(counts.max()), P))
    order = np.argsort(ebin, kind="stable")
    starts = np.zeros(nbins + 1, np.int64)
    starts[1:] = np.cumsum(counts)
    src_a = np.zeros((nbins, P * T), np.int32)
    slot_a = np.full((nbins, P * T), -1.0, np.float32)
    extra_a = None if int_extra is None else np.zeros((nbins, P * T), np.int32)
    for t in range(nbins):
        sel = order[starts[t]:starts[t + 1]]
        cnt = len(sel)
        src_a[t, :cnt] = e_src_pg[sel]
        slot_a[t, :cnt] = eslot[sel]
        if int_extra is not None:
            extra_a[t, :cnt] = int_extra[sel]
    src_a = src_a.reshape(nbins, P, T)
    slot_a = slot_a.reshape(nbins, P, T)
    if extra_a is not None:
        extra_a = extra_a.reshape(nbins, P, T)
    return T, src_a, slot_a, extra_a


def preprocess(x, edge_index, n_nodes, n_cores):
    """Host-side index preprocessing. Returns (cfg dict, per-core data dict)."""
    src = np.asarray(edge_index[0], np.int64)
    dst = np.asarray(edge_index[1], np.int64)
    NPC = n_nodes // n_cores
    NPpad = _ceil(NPC, P) * P
    NT = NPpad // P

    x = np.asarray(x, np.float32)
    owner = dst // NPC

    # degrees for packing: in-degree + 1 (self loop)
    deg = np.bincount(dst, minlength=n_nodes).astype(np.int64) + 1

    ggid = np.zeros(n_nodes, np.int64)   # global -> padded-global permuted id
    pad_perm = []                        # per core: permuted local ids of pad slots
    for k in range(n_cores):
        lo, hi = k * NPC, (k + 1) * NPC
        degs = np.concatenate([deg[lo:hi], np.ones(NPpad - NPC, np.int64)])
        b, s = _pack_bins(degs, NT)
        ggid[lo:hi] = k * NPpad + b[:NPC].astype(np.int64) * P + s[:NPC]
        pad_perm.append(b[NPC:].astype(np.int64) * P + s[NPC:])

    cores = []
    T_gat_all, T_sage_all = 1, 1
    per_core_raw = []
    for k in range(n_cores):
        m = owner == k
        es, ed = src[m], dst[m]
        # GAT edges: + self loops for real locals, + 1 fake edge per pad slot
        sl_nodes = np.arange(k * NPC, (k + 1) * NPC, dtype=np.int64)
        ges = np.concatenate([es, sl_nodes])
        ged = np.concatenate([ed, sl_nodes])
        g_src_pg = ggid[ges]
        g_dstperm = ggid[ged] - k * NPpad
        if len(pad_perm[k]):
            g_src_pg = np.concatenate(
                [g_src_pg, np.full(len(pad_perm[k]), ggid[0], np.int64)])
            g_dstperm = np.concatenate([g_dstperm, pad_perm[k]])
        # SAGE edges: raw edges only
        s_src_pg = ggid[es]
        s_dstperm = ggid[ed] - k * NPpad
        per_core_raw.append((g_src_pg, g_dstperm, s_src_pg, s_dstperm))
        T_gat_all = max(T_gat_all, _ceil(int(np.bincount(
            g_dstperm // P, minlength=NT).max()), P))
        T_sage_all = max(T_sage_all, _ceil(max(1, int(np.bincount(
            s_dstperm // P, minlength=NT).max())), P))

    for k in range(n_cores):
        g_src_pg, g_dstperm, s_src_pg, s_dstperm = per_core_raw[k]
        Tg, gsrc_a, gslot_a, gdst_a = _bucket_edges(
            g_src_pg, g_dstperm, NT, int_extra=g_dstperm)
        Ts, ssrc_a, sslot_a, _ = _bucket_edges(s_src_pg, s_dstperm, NT)
        # pad to uniform T across cores
        if Tg < T_gat_all:
            pad = T_gat_all - Tg
            gsrc_a = np.concatenate([gsrc_a, np.zeros((NT, P, pad), np.int32)], 2)
            gslot_a = np.concatenate([gslot_a, np.full((NT, P, pad), -1.0, np.float32)], 2)
            gdst_a = np.concatenate([gdst_a, np.zeros((NT, P, pad), np.int32)], 2)
        if Ts < T_sage_all:
            pad = T_sage_all - Ts
            ssrc_a = np.concatenate([ssrc_a, np.zeros((NT, P, pad), np.int32)], 2)
            sslot_a = np.concatenate([sslot_a, np.full((NT, P, pad), -1.0, np.float32)], 2)
        # sage deginv per (bin, slot)
        degs = np.bincount(s_dstperm, minlength=NPpad).astype(np.float32)
        deginv = (1.0 / np.maximum(degs, 1.0)).reshape(NT, P, 1)
        # x shard in permuted order
        x_sh = np.zeros((NPpad, x.shape[1]), np.float32)
        lperm = ggid[k * NPC:(k + 1) * NPC] - k * NPpad
        x_sh[lperm] = x[k * NPC:(k + 1) * NPC]
        meta_gat = np.concatenate([gsrc_a, gdst_a], 2).astype(np.int32)  # [NT,P,2T]
        slot_sage = np.concatenate([sslot_a, deginv], 2).astype(np.float32)
        cores.append(dict(
            x_sh=x_sh,
            meta_gat=np.ascontiguousarray(meta_gat),
            slot_gat=np.ascontiguousarray(gslot_a.astype(np.float32)),
            meta_sage=np.ascontiguousarray(ssrc_a.astype(np.int32)),
            slot_sage=np.ascontiguousarray(slot_sage),
        ))

    cfg = dict(n_cores=n_cores, NPC=NPC, NP=NPpad, NT=NT,
               T_gat=T_gat_all, T_sage=T_sage_all, Fin=x.shape[1])
    # host keeps ggid to unpermute outputs
    return cfg, cores, ggid


def fold_weights(W1, a1s, a1d, b1, W2, a2s, a2d, b2, Wl, bl, Wr, M1, mb1, M2, mb2):
    """Host-side weight folding -> replicated device weight arrays."""
    f = lambda a: np.asarray(a, np.float32)
    W1, a1s, a1d, b1 = f(W1), f(a1s), f(a1d), f(b1)
    W2, a2s, a2d, b2 = f(W2), f(a2s), f(a2d), f(b2)
    Wl, bl, Wr, M1, mb1, M2, mb2 = f(Wl), f(bl), f(Wr), f(M1), f(mb1), f(M2), f(mb2)

    def bd(a):  # [HEADS, HID] -> block diag [C, HEADS]
        out = np.zeros((C, HEADS), np.float32)
        for h in range(HEADS):
            out[h * HID:(h + 1) * HID, h] = a[h]
        return out

    w1cat = np.concatenate([W1, W1 @ bd(a1s), W1 @ bd(a1d)], 1)  # [Fin,198]
    w2cat = np.concatenate([W2, W2 @ bd(a2s), W2 @ bd(a2d)], 1)  # [C,198]
    wlmm = Wl @ M1 @ M2                                          # [C,16]
    wrmm = Wr @ M1 @ M2                                          # [C,16]
    cvec = bl @ M1 @ M2 + mb1 @ M2 + mb2                         # [16]
    return dict(
        w1cat=np.ascontiguousarray(w1cat),
        w2cat=np.ascontiguousarray(w2cat),
        wlmm=np.ascontiguousarray(wlmm),
        wrmm=np.ascontiguousarray(wrmm),
        brep1=np.ascontiguousarray(np.tile(b1[None, :], (P, 1))),
        brep2=np.ascontiguousarray(np.tile(b2[None, :], (P, 1))),
        crep=np.ascontiguousarray(np.tile(cvec[None, :], (P, 1))),
    )


def build_program(cfg):
    """Build the Bass/Tile program (SPMD, identical across cores)."""
    import concourse.bass as bass
    import concourse.bacc as bacc
    import concourse.mybir as mybir
    import concourse.tile as tile
    from concourse.masks import make_identity

    n_cores = cfg["n_cores"]
    NP_, NT_, Tg, Ts, Fin = cfg["NP"], cfg["NT"], cfg["T_gat"], cfg["T_sage"], cfg["Fin"]
    NG = n_cores * NP_
    f32 = mybir.dt.float32
    i32 = mybir.dt.int32
    A = mybir.AluOpType
    ACT = mybir.ActivationFunctionType

    nc = bacc.Bacc("TRN2", target_bir_lowering=False, num_devices=n_cores)

    # I/O
    x_in = nc.dram_tensor("x_sh", [NP_, Fin], f32, kind="ExternalInput")
    w1cat = nc.dram_tensor("w1cat", [Fin, C + 2 * HEADS], f32, kind="ExternalInput")
    w2cat = nc.dram_tensor("w2cat", [C, C + 2 * HEADS], f32, kind="ExternalInput")
    wlmm = nc.dram_tensor("wlmm", [C, OUT_C], f32, kind="ExternalInput")
    wrmm = nc.dram_tensor("wrmm", [C, OUT_C], f32, kind="ExternalInput")
    brep1 = nc.dram_tensor("brep1", [P, C], f32, kind="ExternalInput")
    brep2 = nc.dram_tensor("brep2", [P, C], f32, kind="ExternalInput")
    crep = nc.dram_tensor("crep", [P, OUT_C], f32, kind="ExternalInput")
    meta_gat = nc.dram_tensor("meta_gat", [NT_, P, 2 * Tg], i32, kind="ExternalInput")
    slot_gat = nc.dram_tensor("slot_gat", [NT_, P, Tg], f32, kind="ExternalInput")
    meta_sage = nc.dram_tensor("meta_sage", [NT_, P, Ts], i32, kind="ExternalInput")
    slot_sage = nc.dram_tensor("slot_sage", [NT_, P, Ts + 1], f32, kind="ExternalInput")
    out_sh = nc.dram_tensor("out_sh", [NP_, OUT_C], f32, kind="ExternalOutput")

    # Internal DRAM
    g1_loc = nc.dram_tensor("g1_loc", [NP_, ROW], f32, kind="Internal")
    ald1 = nc.dram_tensor("ald1", [NP_, HEADS], f32, kind="Internal")
    f2 = nc.dram_tensor("f2", [NP_, C], f32, kind="Internal")
    g2_loc = nc.dram_tensor("g2_loc", [NP_, ROW], f32, kind="Internal")
    ald2 = nc.dram_tensor("ald2", [NP_, HEADS], f32, kind="Internal")
    f3 = nc.dram_tensor("f3", [NP_, C], f32, kind="Internal")
    if n_cores > 1:
        aspace = "Shared" if n_cores > 4 else "Local"
        g1_full = nc.dram_tensor("g1_full", [NG, ROW], f32, kind="Internal",
                                 addr_space=aspace)
        g2_full = nc.dram_tensor("g2_full", [NG, ROW], f32, kind="Internal",
                                 addr_space=aspace)
        f3_full = nc.dram_tensor("f3_full", [NG, C], f32, kind="Internal",
                                 addr_space=aspace)
    else:
        g1_full, g2_full, f3_full = g1_loc, g2_loc, f3

    NC198 = C + 2 * HEADS  # 198

    with tile.TileContext(nc) as tc:
        import contextlib
        ctx = contextlib.ExitStack()
        with ctx:
            cpool = ctx.enter_context(tc.tile_pool(name="const", bufs=1))
            dpool = ctx.enter_context(tc.tile_pool(name="dense", bufs=3))
            epool = ctx.enter_context(tc.tile_pool(name="edge", bufs=2))
            spool = ctx.enter_context(tc.tile_pool(name="spool", bufs=4))
            accps = ctx.enter_context(tc.tile_pool(name="accps", bufs=4, space="PSUM"))
            trps = ctx.enter_context(tc.tile_pool(name="trps", bufs=2, space="PSUM"))
            ops_ps = ctx.enter_context(tc.tile_pool(name="opsps", bufs=2, space="PSUM"))

            # constants
            iota_i = cpool.tile([P, P], i32)
            iota_f = cpool.tile([P, P], f32)
            ident = cpool.tile([P, P], f32)
            nc.gpsimd.iota(iota_i[:], pattern=[[1, P]], base=0, channel_multiplier=0)
            nc.vector.tensor_copy(iota_f[:], iota_i[:])
            make_identity(nc, ident[:])

            # resident weights
            w1_sb = cpool.tile([Fin, NC198], f32)
            nc.sync.dma_start(w1_sb[:], w1cat[:, :])
            w2a_sb = cpool.tile([P, NC198], f32)
            w2b_sb = cpool.tile([C - P, NC198], f32)
            nc.sync.dma_start(w2a_sb[:], w2cat[0:P, :])
            nc.sync.dma_start(w2b_sb[:], w2cat[P:C, :])
            wl_a = cpool.tile([P, OUT_C], f32)
            wl_b = cpool.tile([C - P, OUT_C], f32)
            wr_a = cpool.tile([P, OUT_C], f32)
            wr_b = cpool.tile([C - P, OUT_C], f32)
            nc.sync.dma_start(wl_a[:], wlmm[0:P, :])
            nc.sync.dma_start(wl_b[:], wlmm[P:C, :])
            nc.sync.dma_start(wr_a[:], wrmm[0:P, :])
            nc.sync.dma_start(wr_b[:], wrmm[P:C, :])
            b1_sb = cpool.tile([P, C], f32)
            b2_sb = cpool.tile([P, C], f32)
            c_sb = cpool.tile([P, OUT_C], f32)
            nc.sync.dma_start(b1_sb[:], brep1[:, :])
            nc.sync.dma_start(b2_sb[:], brep2[:, :])
            nc.sync.dma_start(c_sb[:], crep[:, :])

            def dense_phase(f_dram, Fin_, wblocks, g_dram, ald_dram, scope):
                # wblocks: list of (sb_tile, k0, kw)
                with nc.named_scope(scope):
                    for c in range(NT_):
                        fsb = dpool.tile([P, Fin_], f32, tag="fsb")
                        nc.sync.dma_start(fsb[:], f_dram[c * P:(c + 1) * P, :])
                        gps = accps.tile([P, NC198], f32, tag="acc")
                        nblk = len(wblocks)
                        for bi, (wt, k0, kw) in enumerate(wblocks):
                            tp = trps.tile([P, P], f32, tag="tp")
                            nc.tensor.transpose(out=tp[:kw, :], in_=fsb[:, k0:k0 + kw],
                                                identity=ident[:])
                            ft = dpool.tile([P, P], f32, tag="ft")
                            nc.vector.tensor_copy(ft[:kw, :], tp[:kw, :])
                            nc.tensor.matmul(out=gps[:], lhsT=ft[:kw, :], rhs=wt[:],
                                             start=(bi == 0), stop=(bi == nblk - 1))
                        gsb = dpool.tile([P, NC198], f32, tag="gsb")
                        nc.vector.tensor_copy(gsb[:], gps[:])
                        nc.sync.dma_start(g_dram[c * P:(c + 1) * P, :], gsb[:, 0:ROW])
                        nc.sync.dma_start(ald_dram[c * P:(c + 1) * P, :],
                                          gsb[:, ROW:NC198])

            def allgather(loc, full, scope):
                with nc.named_scope(scope):
                    nc.gpsimd.collective_compute(
                        "AllGather", A.bypass,
                        replica_groups=[list(range(n_cores))],
                        ins=[loc[:, :]],
                        outs=[full[:, :]],
                    )

            def gat_edge_phase(g_full_d, ald_d, b_sb, f_out, scope):
                with nc.named_scope(scope):
                    for t in range(NT_):
                        mi = epool.tile([P, 2 * Tg], i32, tag="mi")
                        nc.sync.dma_start(mi[:], meta_gat[t, :, :])
                        sl = epool.tile([P, Tg], f32, tag="sl")
                        nc.sync.dma_start(sl[:], slot_gat[t, :, :])
                        G = epool.tile([P, Tg, ROW], f32, tag="G")
                        ALD = epool.tile([P, Tg, HEADS], f32, tag="ALD")
                        for j in range(Tg):
                            nc.gpsimd.indirect_dma_start(
                                out=G[:, j, :], out_offset=None, in_=g_full_d[:, :],
                                in_offset=bass.IndirectOffsetOnAxis(
                                    ap=mi[:, j:j + 1], axis=0))
                            nc.gpsimd.indirect_dma_start(
                                out=ALD[:, j, :], out_offset=None, in_=ald_d[:, :],
                                in_offset=bass.IndirectOffsetOnAxis(
                                    ap=mi[:, Tg + j:Tg + j + 1], axis=0))
                        gatt = G[:, :, C:ROW]
                        att = epool.tile([P, Tg, HEADS], f32, tag="att")
                        nc.vector.tensor_tensor(out=att[:], in0=gatt, in1=ALD[:],
                                                op=A.add)
                        e2 = epool.tile([P, Tg, HEADS], f32, tag="e2")
                        nc.scalar.activation(e2[:], att[:], ACT.Exp, scale=0.2)
                        nc.scalar.activation(gatt, att[:], ACT.Exp)
                        nc.vector.tensor_tensor(out=gatt, in0=gatt, in1=e2[:], op=A.max)
                        gh = G[:, :, 0:C].rearrange("p t (h c) -> p t h c", h=HEADS)
                        gw = G[:, :, C:ROW].to_broadcast([P, Tg, HEADS, HID])
                        nc.vector.tensor_tensor(out=gh, in0=gh, in1=gw, op=A.mult)
                        ps = accps.tile([P, ROW], f32, tag="acc")
                        for j in range(Tg):
                            S = spool.tile([P, P], f32, tag="S")
                            nc.vector.tensor_scalar(
                                out=S[:], in0=iota_f[:], scalar1=sl[:, j:j + 1],
                                scalar2=None, op0=A.is_equal)
                            nc.tensor.matmul(out=ps[:], lhsT=S[:], rhs=G[:, j, :],
                                             start=(j == 0), stop=(j == Tg - 1))
                        zinv = epool.tile([P, HEADS], f32, tag="zinv")
                        nc.vector.reciprocal(zinv[:], ps[:, C:ROW])
                        osb = epool.tile([P, C], f32, tag="osb")
                        nc.vector.tensor_tensor(
                            out=osb[:].rearrange("p (h c) -> p h c", h=HEADS),
                            in0=ps[:, 0:C].rearrange("p (h c) -> p h c", h=HEADS),
                            in1=zinv[:].to_broadcast([P, HEADS, HID]),
                            op=A.mult)
                        nc.vector.tensor_tensor(out=osb[:], in0=osb[:], in1=b_sb[:],
                                                op=A.add)
                        nc.scalar.activation(osb[:], osb[:], ACT.Relu)
                        nc.sync.dma_start(f_out[t * P:(t + 1) * P, :], osb[:])

            def sage_phase(scope):
                with nc.named_scope(scope):
                    for t in range(NT_):
                        mi = epool.tile([P, Ts], i32, tag="mi")
                        nc.sync.dma_start(mi[:], meta_sage[t, :, :])
                        sl = epool.tile([P, Ts + 1], f32, tag="sl")
                        nc.sync.dma_start(sl[:], slot_sage[t, :, :])
                        G = epool.tile([P, Ts, C], f32, tag="G")
                        for j in range(Ts):
                            nc.gpsimd.indirect_dma_start(
                                out=G[:, j, :], out_offset=None, in_=f3_full[:, :],
                                in_offset=bass.IndirectOffsetOnAxis(
                                    ap=mi[:, j:j + 1], axis=0))
                        ps = accps.tile([P, C], f32, tag="acc")
                        for j in range(Ts):
                            S = spool.tile([P, P], f32, tag="S")
                            nc.vector.tensor_scalar(
                                out=S[:], in0=iota_f[:], scalar1=sl[:, j:j + 1],
                                scalar2=None, op0=A.is_equal)
                            nc.tensor.matmul(out=ps[:], lhsT=S[:], rhs=G[:, j, :],
                                             start=(j == 0), stop=(j == Ts - 1))
                        asb = epool.tile([P, C], f32, tag="asb")
                        nc.vector.tensor_scalar(out=asb[:], in0=ps[:],
                                                scalar1=sl[:, Ts:Ts + 1], scalar2=None,
                                                op0=A.mult)
                        h2sb = epool.tile([P, C], f32, tag="h2sb")
                        nc.sync.dma_start(h2sb[:], f3[t * P:(t + 1) * P, :])
                        ops = ops_ps.tile([P, OUT_C], f32, tag="ops")
                        blocks = [(asb, wl_a, 0, P), (asb, wl_b, P, C - P),
                                  (h2sb, wr_a, 0, P), (h2sb, wr_b, P, C - P)]
                        for bi, (xsb, wt, k0, kw) in enumerate(blocks):
                            tp = trps.tile([P, P], f32, tag="tp")
                            nc.tensor.transpose(out=tp[:kw, :], in_=xsb[:, k0:k0 + kw],
                                                identity=ident[:])
                            xt = epool.tile([P, P], f32, tag="xt")
                            nc.vector.tensor_copy(xt[:kw, :], tp[:kw, :])
                            nc.tensor.matmul(out=ops[:], lhsT=xt[:kw, :], rhs=wt[:],
                                             start=(bi == 0), stop=(bi == 3))
                        fin = epool.tile([P, OUT_C], f32, tag="fin")
                        nc.vector.tensor_tensor(out=fin[:], in0=ops[:], in1=c_sb[:],
                                                op=A.add)
                        nc.scalar.activation(fin[:], fin[:], ACT.Sigmoid)
                        nc.sync.dma_start(out_sh[t * P:(t + 1) * P, :], fin[:])

            # ---- program ----
            dense_phase(x_in, Fin, [(w1_sb, 0, Fin)], g1_loc, ald1, "dense1")
            if n_cores > 1:
                allgather(g1_loc, g1_full, "ag1")
            gat_edge_phase(g1_full, ald1, b1_sb, f2, "edge1")
            dense_phase(f2, C, [(w2a_sb, 0, P), (w2b_sb, P, C - P)], g2_loc, ald2,
                        "dense2")
            if n_cores > 1:
                allgather(g2_loc, g2_full, "ag2")
            gat_edge_phase(g2_full, ald2, b2_sb, f3, "edge2")
            if n_cores > 1:
                allgather(f3, f3_full, "ag3")
            sage_phase("sage")

    nc.compile()
    return nc


LAST_RESULTS = None  # BassKernelResults of the most recent kernel() call


def kernel(**inputs):
    global LAST_RESULTS
    import os
    x = np.asarray(inputs["x"], np.float32)
    edge_index = np.asarray(inputs["edge_index"])
    cfg, cores, ggid = preprocess(x, edge_index, N, NCORES)
    wts = fold_weights(
        inputs["W1"], inputs["a1s"], inputs["a1d"], inputs["b1"],
        inputs["W2"], inputs["a2s"], inputs["a2d"], inputs["b2"],
        inputs["Wl"], inputs["bl"], inputs["Wr"],
        inputs["M1"], inputs["mb1"], inputs["M2"], inputs["mb2"])
    nc = build_program(cfg)
    in_maps = [dict(core, **wts) for core in cores]

    from concourse import bass_utils
    res = bass_utils.run_bass_kernel_spmd(
        nc, in_maps, core_ids=list(range(NCORES)),
        trace=bool(int(os.environ.get("GAT_TRACE", "0"))))
    LAST_RESULTS = res
    NPp = cfg["NP"]
    out = np.zeros((N, OUT_C), np.float32)
    for k in range(NCORES):
        o = res.results[k]["out_sh"]  # [NP, OUT_C]
        lo, hi = k * cfg["NPC"], (k + 1) * cfg["NPC"]
        out[lo:hi] = o[ggid[lo:hi] - k * NPp]
    return out


# revision 18
# speedup vs baseline: 1.4579x; 1.4579x over previous
"""Distributed GATv1 (2x GAT + SAGE + MLP head) for Trainium2, 8 NeuronCores.

Strategy (graph/data parallel, per sharding hint):
- Nodes are sharded contiguously across the 8 cores; each core's local nodes
  are re-binned into tiles of 128 ("dst bins") balanced by in-degree so every
  bin has nearly the same number of incoming edges.
- Per GAT layer: a sharded dense phase computes g = [h | al_src] and al_dst
  for local nodes, then an AllGather replicates g; the edge phase gathers
  g[src] rows with indirect DMA, computes softmax weights
  w = exp(leaky_relu(al_s + al_d)) per edge (numerically safe without the
  max-subtraction since |logits| are O(1)), scales the gathered rows, and
  aggregates messages per dst bin with a one-hot "routing" matmul that also
  accumulates the softmax denominators as 3 extra columns.
- SAGE mean-aggregation reuses the same machinery with unit weights; its
  linear layers and the whole MLP head collapse into two [192,16] matmuls
  (no nonlinearity in between), folded on the host.
"""

import numpy as np

# Problem constants (hardcoded; kernel.py must be self-contained).
N = 50000
E = 800000
IN_C = 128
HID = 64
HEADS = 3
OUT_C = 16
C = HEADS * HID          # 192
ROW = C + HEADS          # 195 = [h | al_s]
NCORES = 8
P = 128


def _ceil(a, b):
    return -(-a - 0) // b if False else -(-a // b)


def _pack_bins(deg, nbins):
    """Greedy balanced binning: assign n=nbins*128 nodes to bins of 128 slots,
    minimizing the max per-bin edge count. Returns (bin_of, slot_of)."""
    n = len(deg)
    assert n == nbins * P
    order = np.argsort(-deg, kind="stable")
    bin_load = np.zeros(nbins, np.int64)
    bin_fill = np.zeros(nbins, np.int64)
    bin_of = np.zeros(n, np.int32)
    slot_of = np.zeros(n, np.int32)
    big = np.int64(1 << 60)
    for l in order:
        cand = np.where(bin_fill < P, bin_load, big)
        b = int(np.argmin(cand))
        bin_of[l] = b
        slot_of[l] = bin_fill[b]
        bin_fill[b] += 1
        bin_load[b] += deg[l]
    assert (bin_fill == P).all()
    return bin_of, slot_of


def _bucket_edges(e_src_pg, e_dstperm, nbins, int_extra=None):
    """Bucket edges by dst bin into [nbins, P, T] arrays (T = max needed).
    Returns (T, src_a[nbins,P,T] i32, slot_a[nbins,P,T] f32, extra_a or None)."""
    ebin = e_dstperm // P
    eslot = (e_dstperm % P).astype(np.float32)
    counts = np.bincount(ebin, minlength=nbins)
    T = max(1, _ceil(int(counts.max()), P))
    order = np.argsort(ebin, kind="stable")
    starts = np.zeros(nbins + 1, np.int64)
    starts[1:] = np.cumsum(counts)
    src_a = np.zeros((nbins, P * T), np.int32)
    slot_a = np.full((nbins, P * T), -1.0, np.float32)
    extra_a = None if int_extra is None else np.zeros((nbins, P * T), np.int32)
    for t in range(nbins):
        sel = order[starts[t]:starts[t + 1]]
        cnt = len(sel)
        src_a[t, :cnt] = e_src_pg[sel]
        slot_a[t, :cnt] = eslot[sel]
        if int_extra is not None:
            extra_a[t, :cnt] = int_extra[sel]
    src_a = src_a.reshape(nbins, P, T)
    slot_a = slot_a.reshape(nbins, P, T)
    if extra_a is not None:
        extra_a = extra_a.reshape(nbins, P, T)
    return T, src_a, slot_a, extra_a


def preprocess(x, edge_index, n_nodes, n_cores):
    """Host-side index preprocessing. Returns (cfg dict, per-core data dict)."""
    src = np.asarray(edge_index[0], np.int64)
    dst = np.asarray(edge_index[1], np.int64)
    NPC = n_nodes // n_cores
    NPpad = _ceil(NPC, P) * P
    NT = NPpad // P

    x = np.asarray(x, np.float32)
    owner = dst // NPC

    # degrees for packing: in-degree + 1 (self loop)
    deg = np.bincount(dst, minlength=n_nodes).astype(np.int64) + 1

    ggid = np.zeros(n_nodes, np.int64)   # global -> padded-global permuted id
    pad_perm = []                        # per core: permuted local ids of pad slots
    for k in range(n_cores):
        lo, hi = k * NPC, (k + 1) * NPC
        degs = np.concatenate([deg[lo:hi], np.ones(NPpad - NPC, np.int64)])
        b, s = _pack_bins(degs, NT)
        ggid[lo:hi] = k * NPpad + b[:NPC].astype(np.int64) * P + s[:NPC]
        pad_perm.append(b[NPC:].astype(np.int64) * P + s[NPC:])

    cores = []
    T_gat_all, T_sage_all = 1, 1
    per_core_raw = []
    for k in range(n_cores):
        m = owner == k
        es, ed = src[m], dst[m]
        # GAT edges: + self loops for real locals, + 1 fake edge per pad slot
        sl_nodes = np.arange(k * NPC, (k + 1) * NPC, dtype=np.int64)
        ges = np.concatenate([es, sl_nodes])
        ged = np.concatenate([ed, sl_nodes])
        g_src_pg = ggid[ges]
        g_dstperm = ggid[ged] - k * NPpad
        if len(pad_perm[k]):
            g_src_pg = np.concatenate(
                [g_src_pg, np.full(len(pad_perm[k]), ggid[0], np.int64)])
            g_dstperm = np.concatenate([g_dstperm, pad_perm[k]])
        # SAGE edges: raw edges only
        s_src_pg = ggid[es]
        s_dstperm = ggid[ed] - k * NPpad
        per_core_raw.append((g_src_pg, g_dstperm, s_src_pg, s_dstperm))
        T_gat_all = max(T_gat_all, _ceil(int(np.bincount(
            g_dstperm // P, minlength=NT).max()), P))
        T_sage_all = max(T_sage_all, _ceil(max(1, int(np.bincount(
            s_dstperm // P, minlength=NT).max())), P))

    for k in range(n_cores):
        g_src_pg, g_dstperm, s_src_pg, s_dstperm = per_core_raw[k]
        Tg, gsrc_a, gslot_a, gdst_a = _bucket_edges(
            g_src_pg, g_dstperm, NT, int_extra=g_dstperm)
        Ts, ssrc_a, sslot_a, _ = _bucket_edges(s_src_pg, s_dstperm, NT)
        # pad to uniform T across cores
        if Tg < T_gat_all:
            pad = T_gat_all - Tg
            gsrc_a = np.concatenate([gsrc_a, np.zeros((NT, P, pad), np.int32)], 2)
            gslot_a = np.concatenate([gslot_a, np.full((NT, P, pad), -1.0, np.float32)], 2)
            gdst_a = np.concatenate([gdst_a, np.zeros((NT, P, pad), np.int32)], 2)
        if Ts < T_sage_all:
            pad = T_sage_all - Ts
            ssrc_a = np.concatenate([ssrc_a, np.zeros((NT, P, pad), np.int32)], 2)
            sslot_a = np.concatenate([sslot_a, np.full((NT, P, pad), -1.0, np.float32)], 2)
        # sage deginv per (bin, slot)
        degs = np.bincount(s_dstperm, minlength=NPpad).astype(np.float32)
        deginv = (1.0 / np.maximum(degs, 1.0)).reshape(NT, P, 1)
        # x shard in permuted order
        x_sh = np.zeros((NPpad, x.shape[1]), np.float32)
        lperm = ggid[k * NPC:(k + 1) * NPC] - k * NPpad
        x_sh[lperm] = x[k * NPC:(k + 1) * NPC]
        slot_sage = np.concatenate([sslot_a, deginv], 2).astype(np.float32)
        # slot row layout [NT, 1, T*P] for the partition-broadcast matmul
        slot_gat_r = np.ascontiguousarray(
            gslot_a.astype(np.float32).transpose(0, 2, 1).reshape(NT, 1, -1))
        cores.append(dict(
            x_sh=x_sh,
            meta_gat=np.ascontiguousarray(gsrc_a.astype(np.int32)),
            slot_gat=np.ascontiguousarray(gslot_a.astype(np.float32)),
            slot_gat_r=slot_gat_r,
            meta_sage=np.ascontiguousarray(ssrc_a.astype(np.int32)),
            slot_sage=np.ascontiguousarray(slot_sage),
        ))

    cfg = dict(n_cores=n_cores, NPC=NPC, NP=NPpad, NT=NT,
               T_gat=T_gat_all, T_sage=T_sage_all, Fin=x.shape[1])
    # host keeps ggid to unpermute outputs
    return cfg, cores, ggid


def fold_weights(W1, a1s, a1d, b1, W2, a2s, a2d, b2, Wl, bl, Wr, M1, mb1, M2, mb2):
    """Host-side weight folding -> replicated device weight arrays."""
    f = lambda a: np.asarray(a, np.float32)
    W1, a1s, a1d, b1 = f(W1), f(a1s), f(a1d), f(b1)
    W2, a2s, a2d, b2 = f(W2), f(a2s), f(a2d), f(b2)
    Wl, bl, Wr, M1, mb1, M2, mb2 = f(Wl), f(bl), f(Wr), f(M1), f(mb1), f(M2), f(mb2)

    def bd(a):  # [HEADS, HID] -> block diag [C, HEADS]
        out = np.zeros((C, HEADS), np.float32)
        for h in range(HEADS):
            out[h * HID:(h + 1) * HID, h] = a[h]
        return out

    w1cat = np.concatenate([W1, W1 @ bd(a1s), W1 @ bd(a1d)], 1)  # [Fin,198]
    w2cat = np.concatenate([W2, W2 @ bd(a2s), W2 @ bd(a2d)], 1)  # [C,198]
    wlmm = Wl @ M1 @ M2                                          # [C,16]
    wrmm = Wr @ M1 @ M2                                          # [C,16]
    cvec = bl @ M1 @ M2 + mb1 @ M2 + mb2                         # [16]
    return dict(
        w1cat=np.ascontiguousarray(w1cat),
        w2cat=np.ascontiguousarray(w2cat),
        wlmm=np.ascontiguousarray(wlmm),
        wrmm=np.ascontiguousarray(wrmm),
        brep1=np.ascontiguousarray(np.tile(b1[None, :], (P, 1))),
        brep2=np.ascontiguousarray(np.tile(b2[None, :], (P, 1))),
        crep=np.ascontiguousarray(np.tile(cvec[None, :], (P, 1))),
    )


def build_program(cfg):
    """Build the Bass/Tile program (SPMD, identical across cores)."""
    import concourse.bass as bass
    import concourse.bacc as bacc
    import concourse.mybir as mybir
    import concourse.tile as tile
    from concourse.masks import make_identity

    n_cores = cfg["n_cores"]
    NP_, NT_, Tg, Ts, Fin = cfg["NP"], cfg["NT"], cfg["T_gat"], cfg["T_sage"], cfg["Fin"]
    NG = n_cores * NP_
    f32 = mybir.dt.float32
    i32 = mybir.dt.int32
    A = mybir.AluOpType
    ACT = mybir.ActivationFunctionType

    nc = bacc.Bacc("TRN2", target_bir_lowering=False, num_devices=n_cores)

    # I/O
    x_in = nc.dram_tensor("x_sh", [NP_, Fin], f32, kind="ExternalInput")
    w1cat = nc.dram_tensor("w1cat", [Fin, C + 2 * HEADS], f32, kind="ExternalInput")
    w2cat = nc.dram_tensor("w2cat", [C, C + 2 * HEADS], f32, kind="ExternalInput")
    wlmm = nc.dram_tensor("wlmm", [C, OUT_C], f32, kind="ExternalInput")
    wrmm = nc.dram_tensor("wrmm", [C, OUT_C], f32, kind="ExternalInput")
    brep1 = nc.dram_tensor("brep1", [P, C], f32, kind="ExternalInput")
    brep2 = nc.dram_tensor("brep2", [P, C], f32, kind="ExternalInput")
    crep = nc.dram_tensor("crep", [P, OUT_C], f32, kind="ExternalInput")
    meta_gat = nc.dram_tensor("meta_gat", [NT_, P, Tg], i32, kind="ExternalInput")
    slot_gat = nc.dram_tensor("slot_gat", [NT_, P, Tg], f32, kind="ExternalInput")
    slot_gat_r = nc.dram_tensor("slot_gat_r", [NT_, 1, Tg * P], f32,
                                kind="ExternalInput")
    meta_sage = nc.dram_tensor("meta_sage", [NT_, P, Ts], i32, kind="ExternalInput")
    slot_sage = nc.dram_tensor("slot_sage", [NT_, P, Ts + 1], f32, kind="ExternalInput")
    out_sh = nc.dram_tensor("out_sh", [NP_, OUT_C], f32, kind="ExternalOutput")

    # Internal DRAM
    g1_loc = nc.dram_tensor("g1_loc", [NP_, ROW], f32, kind="Internal")
    ald1 = nc.dram_tensor("ald1", [NP_, HEADS], f32, kind="Internal")
    f2 = nc.dram_tensor("f2", [NP_, C], f32, kind="Internal")
    g2_loc = nc.dram_tensor("g2_loc", [NP_, ROW], f32, kind="Internal")
    ald2 = nc.dram_tensor("ald2", [NP_, HEADS], f32, kind="Internal")
    f3 = nc.dram_tensor("f3", [NP_, C], f32, kind="Internal")
    if n_cores > 1:
        aspace = "Shared" if n_cores > 4 else "Local"
        g1_full = nc.dram_tensor("g1_full", [NG, ROW], f32, kind="Internal",
                                 addr_space=aspace)
        g2_full = nc.dram_tensor("g2_full", [NG, ROW], f32, kind="Internal",
                                 addr_space=aspace)
        f3_full = nc.dram_tensor("f3_full", [NG, C], f32, kind="Internal",
                                 addr_space=aspace)
    else:
        g1_full, g2_full, f3_full = g1_loc, g2_loc, f3

    NC198 = C + 2 * HEADS  # 198

    with tile.TileContext(nc) as tc:
        import contextlib
        ctx = contextlib.ExitStack()
        with ctx:
            cpool = ctx.enter_context(tc.tile_pool(name="const", bufs=1))
            dpool = ctx.enter_context(tc.tile_pool(name="dense", bufs=3))
            epool = ctx.enter_context(tc.tile_pool(name="edge", bufs=2))
            spool = ctx.enter_context(tc.tile_pool(name="spool", bufs=2))
            accps = ctx.enter_context(tc.tile_pool(name="accps", bufs=3, space="PSUM"))
            trps = ctx.enter_context(tc.tile_pool(name="trps", bufs=2, space="PSUM"))
            ops_ps = ctx.enter_context(tc.tile_pool(name="opsps", bufs=2, space="PSUM"))

            # constants
            iota_i = cpool.tile([P, P], i32)
            iota_f = cpool.tile([P, P], f32)
            ident = cpool.tile([P, P], f32)
            nc.gpsimd.iota(iota_i[:], pattern=[[1, P]], base=0, channel_multiplier=0)
            nc.vector.tensor_copy(iota_f[:], iota_i[:])
            make_identity(nc, ident[:])
            # partition-index tile (value = partition id, const along free)
            ipt_i = cpool.tile([P, 1], i32)
            ipt_f = cpool.tile([P, 1], f32)
            nc.gpsimd.iota(ipt_i[:], pattern=[[0, 1]], base=0, channel_multiplier=1)
            nc.vector.tensor_copy(ipt_f[:], ipt_i[:])
            ones_sb = cpool.tile([1, P], f32)
            nc.vector.memset(ones_sb[:], 1.0)

            # resident weights
            w1_sb = cpool.tile([Fin, NC198], f32)
            nc.sync.dma_start(w1_sb[:], w1cat[:, :])
            w2a_sb = cpool.tile([P, NC198], f32)
            w2b_sb = cpool.tile([C - P, NC198], f32)
            nc.sync.dma_start(w2a_sb[:], w2cat[0:P, :])
            nc.sync.dma_start(w2b_sb[:], w2cat[P:C, :])
            wl_a = cpool.tile([P, OUT_C], f32)
            wl_b = cpool.tile([C - P, OUT_C], f32)
            wr_a = cpool.tile([P, OUT_C], f32)
            wr_b = cpool.tile([C - P, OUT_C], f32)
            nc.sync.dma_start(wl_a[:], wlmm[0:P, :])
            nc.sync.dma_start(wl_b[:], wlmm[P:C, :])
            nc.sync.dma_start(wr_a[:], wrmm[0:P, :])
            nc.sync.dma_start(wr_b[:], wrmm[P:C, :])
            b1_sb = cpool.tile([P, C], f32)
            b2_sb = cpool.tile([P, C], f32)
            c_sb = cpool.tile([P, OUT_C], f32)
            nc.sync.dma_start(b1_sb[:], brep1[:, :])
            nc.sync.dma_start(b2_sb[:], brep2[:, :])
            nc.sync.dma_start(c_sb[:], crep[:, :])

            def dense_phase(f_dram, Fin_, wblocks, g_dram, ald_dram, scope):
                # wblocks: list of (sb_tile, k0, kw)
                with nc.named_scope(scope):
                    for c in range(NT_):
                        fsb = dpool.tile([P, Fin_], f32, tag="fsb")
                        nc.sync.dma_start(fsb[:], f_dram[c * P:(c + 1) * P, :])
                        gps = accps.tile([P, NC198], f32, tag="acc")
                        nblk = len(wblocks)
                        for bi, (wt, k0, kw) in enumerate(wblocks):
                            tp = trps.tile([P, P], f32, tag="tp")
                            nc.tensor.transpose(out=tp[:kw, :], in_=fsb[:, k0:k0 + kw],
                                                identity=ident[:])
                            ft = dpool.tile([P, P], f32, tag="ft")
                            nc.vector.tensor_copy(ft[:kw, :], tp[:kw, :])
                            nc.tensor.matmul(out=gps[:], lhsT=ft[:kw, :], rhs=wt[:],
                                             start=(bi == 0), stop=(bi == nblk - 1))
                        gsb = dpool.tile([P, NC198], f32, tag="gsb")
                        nc.vector.tensor_copy(gsb[:], gps[:])
                        nc.sync.dma_start(g_dram[c * P:(c + 1) * P, :], gsb[:, 0:ROW])
                        nc.sync.dma_start(ald_dram[c * P:(c + 1) * P, :],
                                          gsb[:, ROW:NC198])

            def allgather(loc, full, scope):
                with nc.named_scope(scope):
                    nc.gpsimd.collective_compute(
                        "AllGather", A.bypass,
                        replica_groups=[list(range(n_cores))],
                        ins=[loc[:, :]],
                        outs=[full[:, :]],
                    )

            def gat_edge_phase(g_full_d, ald_d, b_sb, f_out, scope):
                with nc.named_scope(scope):
                    for t in range(NT_):
                        mi = epool.tile([P, Tg], i32, tag="mi")
                        nc.sync.dma_start(mi[:], meta_gat[t, :, :])
                        sl = epool.tile([P, Tg], f32, tag="sl")
                        nc.sync.dma_start(sl[:], slot_gat[t, :, :])
                        slr = epool.tile([1, Tg * P], f32, tag="slr")
                        nc.sync.dma_start(slr[:], slot_gat_r[t, :, :])
                        aldt = epool.tile([P, HEADS], f32, tag="aldt")
                        nc.sync.dma_start(aldt[:], ald_d[t * P:(t + 1) * P, :])
                        G = epool.tile([P, Tg, ROW], f32, tag="G")
                        for j in range(Tg):
                            nc.gpsimd.indirect_dma_start(
                                out=G[:, j, :], out_offset=None, in_=g_full_d[:, :],
                                in_offset=bass.IndirectOffsetOnAxis(
                                    ap=mi[:, j:j + 1], axis=0))
                        # batched one-hot S for all edge tiles: S_all[e,j,d]
                        S_all = spool.tile([P, Tg, P], f32, tag="S")
                        nc.vector.tensor_tensor(
                            out=S_all[:],
                            in0=iota_f[:].unsqueeze(1).broadcast_to([P, Tg, P]),
                            in1=sl[:].unsqueeze(2).broadcast_to([P, Tg, P]),
                            op=A.is_equal)
                        ps = accps.tile([P, ROW], f32, tag="acc")
                        for j in range(Tg):
                            # al_d routing: S_T[d,e] = (d == slot_e) via
                            # partition-broadcast matmul + is_equal, then
                            # alde[e,:] = S_T.T @ aldt
                            br = trps.tile([P, P], f32, tag="tp")
                            nc.tensor.matmul(out=br[:], lhsT=ones_sb[:],
                                             rhs=slr[:, j * P:(j + 1) * P],
                                             start=True, stop=True)
                            st = epool.tile([P, P], f32, tag="st")
                            nc.vector.tensor_tensor(
                                out=st[:],
                                in0=ipt_f[:].broadcast_to([P, P]),
                                in1=br[:], op=A.is_equal)
                            alde = ops_ps.tile([P, HEADS], f32, tag="small")
                            nc.tensor.matmul(out=alde[:], lhsT=st[:], rhs=aldt[:],
                                             start=True, stop=True)
                            gatt = G[:, j, C:ROW]
                            att = epool.tile([P, HEADS], f32, tag="att")
                            nc.vector.tensor_tensor(out=att[:], in0=gatt,
                                                    in1=alde[:], op=A.add)
                            e2 = epool.tile([P, HEADS], f32, tag="e2")
                            nc.scalar.activation(e2[:], att[:], ACT.Exp, scale=0.2)
                            nc.scalar.activation(gatt, att[:], ACT.Exp)
                            nc.vector.tensor_tensor(out=gatt, in0=gatt, in1=e2[:],
                                                    op=A.max)
                            gh = G[:, j, 0:C].rearrange("p (h c) -> p h c", h=HEADS)
                            gw = G[:, j, C:ROW].to_broadcast([P, HEADS, HID])
                            nc.vector.tensor_tensor(out=gh, in0=gh, in1=gw, op=A.mult)
                            nc.tensor.matmul(out=ps[:], lhsT=S_all[:, j, :],
                                             rhs=G[:, j, :],
                                             start=(j == 0), stop=(j == Tg - 1))
                        zinv = epool.tile([P, HEADS], f32, tag="zinv")
                        nc.vector.reciprocal(zinv[:], ps[:, C:ROW])
                        osb = epool.tile([P, C], f32, tag="osb")
                        nc.vector.tensor_tensor(
                            out=osb[:].rearrange("p (h c) -> p h c", h=HEADS),
                            in0=ps[:, 0:C].rearrange("p (h c) -> p h c", h=HEADS),
                            in1=zinv[:].to_broadcast([P, HEADS, HID]),
                            op=A.mult)
                        nc.vector.tensor_tensor(out=osb[:], in0=osb[:], in1=b_sb[:],
                                                op=A.add)
                        nc.scalar.activation(osb[:], osb[:], ACT.Relu)
                        nc.sync.dma_start(f_out[t * P:(t + 1) * P, :], osb[:])

            def sage_phase(scope):
                with nc.named_scope(scope):
                    for t in range(NT_):
                        mi = epool.tile([P, Ts], i32, tag="mi")
                        nc.sync.dma_start(mi[:], meta_sage[t, :, :])
                        sl = epool.tile([P, Ts + 1], f32, tag="sl")
                        nc.sync.dma_start(sl[:], slot_sage[t, :, :])
                        G = epool.tile([P, Ts, C], f32, tag="G")
                        for j in range(Ts):
                            nc.gpsimd.indirect_dma_start(
                                out=G[:, j, :], out_offset=None, in_=f3_full[:, :],
                                in_offset=bass.IndirectOffsetOnAxis(
                                    ap=mi[:, j:j + 1], axis=0))
                        S_all = spool.tile([P, Ts, P], f32, tag="S")
                        nc.vector.tensor_tensor(
                            out=S_all[:],
                            in0=iota_f[:].unsqueeze(1).broadcast_to([P, Ts, P]),
                            in1=sl[:, 0:Ts].unsqueeze(2).broadcast_to([P, Ts, P]),
                            op=A.is_equal)
                        ps = accps.tile([P, C], f32, tag="acc")
                        for j in range(Ts):
                            nc.tensor.matmul(out=ps[:], lhsT=S_all[:, j, :],
                                             rhs=G[:, j, :],
                                             start=(j == 0), stop=(j == Ts - 1))
                        asb = epool.tile([P, C], f32, tag="asb")
                        nc.vector.tensor_scalar(out=asb[:], in0=ps[:],
                                                scalar1=sl[:, Ts:Ts + 1], scalar2=None,
                                                op0=A.mult)
                        h2sb = epool.tile([P, C], f32, tag="h2sb")
                        nc.sync.dma_start(h2sb[:], f3[t * P:(t + 1) * P, :])
                        ops = ops_ps.tile([P, OUT_C], f32, tag="small")
                        blocks = [(asb, wl_a, 0, P), (asb, wl_b, P, C - P),
                                  (h2sb, wr_a, 0, P), (h2sb, wr_b, P, C - P)]
                        for bi, (xsb, wt, k0, kw) in enumerate(blocks):
                            tp = trps.tile([P, P], f32, tag="tp")
                            nc.tensor.transpose(out=tp[:kw, :], in_=xsb[:, k0:k0 + kw],
                                                identity=ident[:])
                            xt = epool.tile([P, P], f32, tag="xt")
                            nc.vector.tensor_copy(xt[:kw, :], tp[:kw, :])
                            nc.tensor.matmul(out=ops[:], lhsT=xt[:kw, :], rhs=wt[:],
                                             start=(bi == 0), stop=(bi == 3))
                        fin = epool.tile([P, OUT_C], f32, tag="fin")
                        nc.vector.tensor_tensor(out=fin[:], in0=ops[:], in1=c_sb[:],
                                                op=A.add)
                        nc.scalar.activation(fin[:], fin[:], ACT.Sigmoid)
                        nc.sync.dma_start(out_sh[t * P:(t + 1) * P, :], fin[:])

            # ---- program ----
            dense_phase(x_in, Fin, [(w1_sb, 0, Fin)], g1_loc, ald1, "dense1")
            if n_cores > 1:
                allgather(g1_loc, g1_full, "ag1")
            gat_edge_phase(g1_full, ald1, b1_sb, f2, "edge1")
            dense_phase(f2, C, [(w2a_sb, 0, P), (w2b_sb, P, C - P)], g2_loc, ald2,
                        "dense2")
            if n_cores > 1:
                allgather(g2_loc, g2_full, "ag2")
            gat_edge_phase(g2_full, ald2, b2_sb, f3, "edge2")
            if n_cores > 1:
                allgather(f3, f3_full, "ag3")
            sage_phase("sage")

    nc.compile()
    return nc


LAST_RESULTS = None  # BassKernelResults of the most recent kernel() call


def kernel(**inputs):
    global LAST_RESULTS
    import os
    x = np.asarray(inputs["x"], np.float32)
    edge_index = np.asarray(inputs["edge_index"])
    cfg, cores, ggid = preprocess(x, edge_index, N, NCORES)
    wts = fold_weights(
        inputs["W1"], inputs["a1s"], inputs["a1d"], inputs["b1"],
        inputs["W2"], inputs["a2s"], inputs["a2d"], inputs["b2"],
        inputs["Wl"], inputs["bl"], inputs["Wr"],
        inputs["M1"], inputs["mb1"], inputs["M2"], inputs["mb2"])
    nc = build_program(cfg)
    in_maps = [dict(core, **wts) for core in cores]

    from concourse import bass_utils
    res = bass_utils.run_bass_kernel_spmd(
        nc, in_maps, core_ids=list(range(NCORES)),
        trace=bool(int(os.environ.get("GAT_TRACE", "0"))))
    LAST_RESULTS = res
    NPp = cfg["NP"]
    out = np.zeros((N, OUT_C), np.float32)
    for k in range(NCORES):
        o = res.results[k]["out_sh"]  # [NP, OUT_C]
        lo, hi = k * cfg["NPC"], (k + 1) * cfg["NPC"]
        out[lo:hi] = o[ggid[lo:hi] - k * NPp]
    return out
